# revision 11
# baseline (speedup 1.0000x reference)
"""Trainium2 Bass kernel for the Roost-style GNN (nn_DescriptorNetworkTorch).

Data-parallel over graphs: 256 graphs of 16 fully-connected atoms sharded as
32 graphs per NeuronCore across 8 cores; params replicated; no collectives.

v2 restructure (vs the exp/min/relu baseline):
  selu(y) = LAM*ALPHA*(e^m - 1) + LAM*(y - m)  with  m = min(y, 0)
  - m computed in ONE DVE pass from PSUM (or, for 3 of 12 families, as
    m' = relu(-y) on the Scalar engine to balance engine load)
  - v = e^m via wide-batched Scalar ACTIVATE over an SBUF super-tile
  - the linear  LAM*W2*y  term folds into a precomputed (W1@W2) matmul on
    the PE (reading the pair tile), so no relu tiles exist at all
  - W2 contracts stacked [v; m] (K=512) + linear + a K=5 rider that carries
    pow*ln(w) (hi/lo bf16 split), the softmax diagonal mask, and all scalar
    constants (b2, -LAM*ALPHA*sum(W2), LAM*b1@W2), assembled on host
  - edges processed in superblocks of 1024 (4 graphs) with 2-bank PSUM
    tiles so every elementwise instruction runs at 1024 free-size
  - softmax: z = exp(gate) in bf16, segmented reduces on DVE
  - pair tiles built by the (otherwise idle) GpSimd engine
"""

import numpy as np

G, K, F, EMB, HID, L, H = 256, 16, 64, 200, 256, 3, 3
NCORES = 8
GPC = G // NCORES          # graphs per core (32)
N = GPC * K                # nodes per core (512)
E = GPC * K * K            # all-pair edges per core (8192)
SB = 8                     # superblocks per layer (1024 edges each)
SBE = 1024
NT = 12                    # hid families: t = h*4 + mlp*2 + half
NDM = 9                    # families 0..8: m on DVE; 9..11: m' = relu(-y) on Scalar
LAM = 1.0507009873554804934193349852946
ALPHA = 1.6732632423543772848170429916717
MASKNEG = -1e30

_PROGRAM_CACHE = {}


def _build_program():
    import concourse.bass as bass
    import concourse.bacc as bacc
    import concourse.mybir as mybir
    import concourse.tile as tile

    dt = mybir.dt
    AF = mybir.ActivationFunctionType
    ALU = mybir.AluOpType
    AX = mybir.AxisListType
    f32 = dt.float32
    bf16 = dt.bfloat16

    nc = bacc.Bacc("TRN2", target_bir_lowering=False, debug=False,
                   num_devices=NCORES)

    # ---------------- DRAM I/O ----------------
    d_eft = nc.dram_tensor("eft", [EMB, N], bf16, kind="ExternalInput")
    d_wi = nc.dram_tensor("wipack", [128, 126], bf16, kind="ExternalInput")
    d_binit = nc.dram_tensor("binit", [63, 1], f32, kind="ExternalInput")
    d_wrow = nc.dram_tensor("wrow", [1, N], f32, kind="ExternalInput")
    d_re5 = nc.dram_tensor("riderE5", [5, E], bf16, kind="ExternalInput")
    d_rc5 = nc.dram_tensor("riderC5", [5, N], bf16, kind="ExternalInput")
    d_w1, d_b1, d_wms, d_wgs, d_wlin, d_rid = [], [], [], [], [], []
    for l in range(L):
        d_w1.append(nc.dram_tensor(f"w1pack{l}", [128, 1536], bf16, kind="ExternalInput"))
        d_b1.append(nc.dram_tensor(f"b1pack{l}", [128, 24], f32, kind="ExternalInput"))
        d_wms.append(nc.dram_tensor(f"wms{l}", [128, 768], bf16, kind="ExternalInput"))
        d_wgs.append(nc.dram_tensor(f"wgs{l}", [128, 768], bf16, kind="ExternalInput"))
        d_wlin.append(nc.dram_tensor(f"wlin{l}", [128, 384], bf16, kind="ExternalInput"))
        d_rid.append(nc.dram_tensor(f"rid{l}", [5, 384], bf16, kind="ExternalInput"))
    d_cw1 = nc.dram_tensor("cw1pack", [128, 1536], bf16, kind="ExternalInput")
    d_cb1 = nc.dram_tensor("cb1pack", [128, 24], f32, kind="ExternalInput")
    d_cwms = nc.dram_tensor("cwms", [128, 768], bf16, kind="ExternalInput")
    d_cwgs = nc.dram_tensor("cwgs", [128, 768], bf16, kind="ExternalInput")
    d_cwlin = nc.dram_tensor("cwlin", [128, 384], bf16, kind="ExternalInput")
    d_crid = nc.dram_tensor("crid", [5, 384], bf16, kind="ExternalInput")
    d_out = nc.dram_tensor("out", [F, GPC], f32, kind="ExternalOutput")

    with tile.TileContext(nc) as tc:
        with tc.tile_pool(name="const", bufs=1) as cp, \
             tc.tile_pool(name="fea", bufs=2) as fp, \
             tc.tile_pool(name="sup", bufs=2) as sp, \
             tc.tile_pool(name="zp", bufs=2) as zp, \
             tc.tile_pool(name="node", bufs=1) as np_, \
             tc.tile_pool(name="pre_ps", bufs=2, space="PSUM") as pps, \
             tc.tile_pool(name="w2_ps", bufs=2, space="PSUM") as wps:

            # ---- load constants ----
            wi = cp.tile([128, 126], bf16, tag="wi")
            nc.sync.dma_start(wi[:], d_wi[:])
            binit = cp.tile([63, 1], f32, tag="binit")
            nc.sync.dma_start(binit[:], d_binit[:])
            ef1 = cp.tile([128, N], bf16, tag="ef1")
            ef2 = cp.tile([72, N], bf16, tag="ef2")
            nc.scalar.dma_start(ef1[0:48, :], d_eft[0:48, :])
            nc.gpsimd.dma_start(ef1[48:92, :], d_eft[48:92, :])
            nc.sync.dma_start(ef1[92:128, :], d_eft[92:128, :])
            nc.scalar.dma_start(ef2[:], d_eft[128:200, :])
            wrow = cp.tile([1, N], f32, tag="wrow")
            nc.sync.dma_start(wrow[:], d_wrow[:])
            w1s, b1s, wmss, wgss, wlins, rids = [], [], [], [], [], []
            for l in range(L):
                t = cp.tile([128, 1536], bf16, tag=f"w1_{l}")
                if l == 0:
                    nc.gpsimd.dma_start(t[:, 0:512], d_w1[l][:, 0:512])
                    nc.scalar.dma_start(t[:, 512:1024], d_w1[l][:, 512:1024])
                    nc.sync.dma_start(t[:, 1024:1536], d_w1[l][:, 1024:1536])
                else:
                    nc.sync.dma_start(t[:], d_w1[l][:])
                w1s.append(t)
                t = cp.tile([128, 24], f32, tag=f"b1_{l}")
                nc.sync.dma_start(t[:], d_b1[l][:]); b1s.append(t)
                t = cp.tile([128, 768], bf16, tag=f"wms_{l}")
                (nc.gpsimd if l == 0 else nc.sync).dma_start(t[:], d_wms[l][:]); wmss.append(t)
                t = cp.tile([128, 768], bf16, tag=f"wgs_{l}")
                (nc.scalar if l == 0 else nc.sync).dma_start(t[:], d_wgs[l][:]); wgss.append(t)
                t = cp.tile([128, 384], bf16, tag=f"wlin_{l}")
                nc.sync.dma_start(t[:], d_wlin[l][:]); wlins.append(t)
                t = cp.tile([5, 384], bf16, tag=f"rid_{l}")
                nc.sync.dma_start(t[:], d_rid[l][:]); rids.append(t)
            re5 = cp.tile([5, E], bf16, tag="re5")
            nc.sync.dma_start(re5[:], d_re5[:])
            rc5 = cp.tile([5, N], bf16, tag="rc5")
            nc.sync.dma_start(rc5[:], d_rc5[:])
            cw1 = cp.tile([128, 1536], bf16, tag="cw1")
            nc.sync.dma_start(cw1[:], d_cw1[:])
            cb1 = cp.tile([128, 24], f32, tag="cb1")
            nc.sync.dma_start(cb1[:], d_cb1[:])
            cwms = cp.tile([128, 768], bf16, tag="cwms")
            nc.sync.dma_start(cwms[:], d_cwms[:])
            cwgs = cp.tile([128, 768], bf16, tag="cwgs")
            nc.sync.dma_start(cwgs[:], d_cwgs[:])
            cwlin = cp.tile([128, 384], bf16, tag="cwlin")
            nc.sync.dma_start(cwlin[:], d_cwlin[:])
            crid = cp.tile([5, 384], bf16, tag="crid")
            nc.sync.dma_start(crid[:], d_crid[:])

            # ---- initial embed ----
            fea = fp.tile([128, N], f32, tag="fea")
            for c in range(N // 512):
                sl = slice(c * 512, (c + 1) * 512)
                emb_ps = wps.tile([63, 512], f32, tag="w2")
                nc.tensor.matmul(emb_ps[:], (wi[0:128, 0:63]), (ef1[:, sl]),
                                 start=True, stop=False)
                nc.tensor.matmul(emb_ps[:], (wi[0:72, 63:126]), (ef2[:, sl]),
                                 start=False, stop=True)
                nc.scalar.activation(fea[0:63, sl], emb_ps[:], AF.Identity,
                                     bias=binit[:], scale=1.0)
            nc.sync.dma_start(fea[63:64, :], wrow[:])
            nc.sync.dma_start(fea[64:128, :], fea[0:64, :])

            U_OF_T = {0: 0, 1: 1, 4: 2, 5: 3, 2: 4, 3: 5, 6: 6, 7: 7,
                      8: 8, 9: 9, 10: 10, 11: 11}

            def mslice(t, nh, sbe):
                return ("M", U_OF_T[t] * sbe + nh * 512)

            def emit_front(sbi, fea_t, w1t, b1t, state, sbe, kdim):
                """pair build + W1 matmuls + m/m' for superblock sbi."""
                pair = sp.tile([128, sbe], bf16, tag="pair", bufs=3)
                if kdim == 128:
                    colb = sbi * (sbe // K)
                    ng = sbe // 256
                    self_src = (fea_t[0:64, colb:colb + ng * K]
                                .rearrange("p (g i) -> p g i", g=ng)
                                .unsqueeze(3).broadcast_to([64, ng, K, K]))
                    nbr_src = (fea_t[64:128, colb:colb + ng * K]
                               .rearrange("p (g j) -> p g j", g=ng)
                               .unsqueeze(2).broadcast_to([64, ng, K, K]))
                    nc.vector.tensor_copy(
                        pair[0:64, :].rearrange("p (g i j) -> p g i j", g=ng, i=K),
                        self_src)
                    nc.scalar.activation(
                        pair[64:128, :].rearrange("p (g i j) -> p g i j", g=ng, i=K),
                        nbr_src, AF.Identity)
                else:
                    nc.vector.tensor_copy(pair[0:64, :], fea_t[0:64, :])
                mAB = sp.tile([128, NT * sbe], bf16, tag="mAB", bufs=3)
                for t in sorted(range(NT), key=lambda t: U_OF_T[t]):
                    u = U_OF_T[t]
                    pre = pps.tile([128, sbe], f32, tag="pre")
                    for nh in range(sbe // 512):
                        nc.tensor.matmul(pre[:, nh * 512:nh * 512 + 512],
                                         (w1t[0:kdim, t * 128:t * 128 + 128]),
                                         (pair[0:kdim, nh * 512:nh * 512 + 512]),
                                         start=True, stop=True)
                    if t < NDM:
                        nc.vector.tensor_scalar(
                            mAB[:, u * sbe:(u + 1) * sbe], pre[:],
                            b1t[:, t:t + 1], 0.0, op0=ALU.add, op1=ALU.min)
                    else:
                        nc.scalar.activation(
                            mAB[:, u * sbe:(u + 1) * sbe], pre[:],
                            AF.Relu, bias=b1t[:, 12 + (t - NDM):13 + (t - NDM)],
                            scale=-1.0)
                state[sbi] = (pair, mAB, None)

            def emit_vs(sbi, state, sbe):
                """wide-batched v = e^m for superblock sbi."""
                pair, mAB, _ = state[sbi]
                vS = sp.tile([128, NT * sbe], bf16, tag="vS", bufs=2)
                nc.scalar.activation(vS[:, 0:4 * sbe], mAB[:, 0:4 * sbe],
                                     AF.Exp)
                nc.scalar.activation(vS[:, 4 * sbe:8 * sbe],
                                     mAB[:, 4 * sbe:8 * sbe], AF.Exp)
                nc.scalar.activation(vS[:, 8 * sbe:9 * sbe],
                                     mAB[:, 8 * sbe:9 * sbe], AF.Exp)
                nc.scalar.activation(vS[:, 9 * sbe:12 * sbe],
                                     mAB[:, 9 * sbe:12 * sbe], AF.Exp,
                                     scale=-1.0)
                state[sbi] = (pair, mAB, vS)

            def emit_back(sbi, state, rider, wmst, wgst, wlint, ridt,
                          dn01, rn01, dn2, rn2, sbe, kdim, fin=None):
                """W2 matmuls + softmax + segmented reduces for superblock sbi."""
                pair, mAB, vS = state.pop(sbi)
                nhs = sbe // 512

                def rhs(kind, off):
                    src = mAB if kind == "M" else vS
                    return src[:, off:off + 512]

                def stacked(ps, h, mlp, base, tp):
                    """8 accumulating chunk matmuls for head h of mlp into
                    row-block tp of ps (per nh bank)."""
                    rows = slice(64, 128) if tp else slice(0, 64)
                    wt = wgst if mlp == 0 else wmst
                    for nh in range(nhs):
                        for k in range(4):
                            half = k % 2
                            t = h * 4 + mlp * 2 + half
                            if k < 2:
                                kind, off = "V", U_OF_T[t] * sbe + nh * 512
                            else:
                                kind, off = mslice(t, nh, sbe)
                            last = (k == 3)
                            nc.tensor.matmul(
                                ps[rows, nh * 512:nh * 512 + 512],
                                (wt[:, h * 256 + k * 64:h * 256 + k * 64 + 64]),
                                (rhs(kind, off)),
                                start=False, stop=(last and base),
                                tile_position=((0, 64) if tp else None),
                                skip_group_check=True)

                def headpair(rcol, lcol, hs, mlps, esl0):
                    """one W2 PSUM tile: rider + linear + 2 row-blocks."""
                    ps = wps.tile([128, sbe], f32, tag="w2")
                    for nh in range(nhs):
                        esl = slice(esl0 + sbi * sbe + nh * 512,
                                    esl0 + sbi * sbe + nh * 512 + 512)
                        nc.tensor.matmul(ps[:, nh * 512:nh * 512 + 512],
                                         (ridt[0:5, rcol:rcol + 128]),
                                         (rider[:, esl]),
                                         start=True, stop=False,
                                         skip_group_check=True)
                        nc.tensor.matmul(ps[:, nh * 512:nh * 512 + 512],
                                         (wlint[0:kdim, lcol:lcol + 128]),
                                         (pair[0:kdim, nh * 512:nh * 512 + 512]),
                                         start=False, stop=False,
                                         skip_group_check=True)
                    stacked(ps, hs[0], mlps[0], False, False)
                    stacked(ps, hs[1], mlps[1], True, True)
                    return ps

                gate01 = headpair(0, 0, (0, 1), (0, 0), 0)
                msg01 = headpair(128, 128, (0, 1), (1, 1), 0)
                g2m2 = headpair(256, 256, (2, 2), (1, 0), 0)

                nseg = sbe // K
                seg = slice(sbi * nseg, (sbi + 1) * nseg)
                z01 = zp.tile([128, sbe], bf16, tag="z01")
                nc.scalar.activation(z01[:], gate01[:], AF.Exp)
                nc.vector.tensor_reduce(
                    out=dn01[:, seg],
                    in_=z01[:].rearrange("p (s j) -> p s j", j=K),
                    axis=AX.X, op=ALU.add)
                prod01 = zp.tile([128, sbe], bf16, tag="prod01")
                nc.vector.tensor_tensor(out=prod01[:], in0=msg01[:], in1=z01[:],
                                        op=ALU.mult)
                nc.vector.tensor_reduce(
                    out=rn01[:, seg],
                    in_=prod01[:].rearrange("p (s j) -> p s j", j=K),
                    axis=AX.X, op=ALU.add)
                z2 = zp.tile([64, sbe], bf16, tag="z2")
                nc.scalar.activation(z2[:], g2m2[64:128, :], AF.Exp)
                nc.vector.tensor_reduce(
                    out=dn2[:, seg],
                    in_=z2[:].rearrange("p (s j) -> p s j", j=K),
                    axis=AX.X, op=ALU.add)
                prod2 = zp.tile([64, sbe], bf16, tag="prod2")
                nc.vector.tensor_tensor(out=prod2[:], in0=g2m2[0:64, :],
                                        in1=z2[:], op=ALU.mult)
                nc.vector.tensor_reduce(
                    out=rn2[:, seg],
                    in_=prod2[:].rearrange("p (s j) -> p s j", j=K),
                    axis=AX.X, op=ALU.add)
                if fin is not None:
                    fea_src, fea_dst = fin
                    nc.vector.reciprocal(dn01[:, seg], dn01[:, seg])
                    nc.vector.reciprocal(dn2[:, seg], dn2[:, seg])
                    nc.vector.tensor_tensor(out=rn01[:, seg], in0=rn01[:, seg],
                                            in1=dn01[:, seg], op=ALU.mult)
                    nc.vector.tensor_tensor(out=rn2[:, seg], in0=rn2[:, seg],
                                            in1=dn2[:, seg], op=ALU.mult)
                    u1lo = np_.tile([64, nseg], f32, tag="u1lo", bufs=3)
                    nc.sync.dma_start(u1lo[:], rn01[64:128, seg])
                    nc.gpsimd.tensor_tensor(out=rn2[:, seg], in0=rn2[:, seg],
                                            in1=u1lo[:], op=ALU.add)
                    nc.gpsimd.tensor_tensor(out=rn2[:, seg], in0=rn2[:, seg],
                                            in1=rn01[0:64, seg], op=ALU.add)
                    nc.gpsimd.tensor_tensor(out=fea_dst[0:64, seg],
                                            in0=fea_src[0:64, seg],
                                            in1=rn2[:, seg], op=ALU.add)
                    nc.sync.dma_start(fea_dst[64:128, seg], fea_dst[0:64, seg])

            def finish_update(dn01, rn01, dn2, rn2, nseg):
                nc.vector.reciprocal(dn01[:], dn01[:])
                nc.vector.reciprocal(dn2[:], dn2[:])
                nc.vector.tensor_tensor(out=rn01[:], in0=rn01[:], in1=dn01[:],
                                        op=ALU.mult)
                nc.vector.tensor_tensor(out=rn2[:], in0=rn2[:], in1=dn2[:],
                                        op=ALU.mult)
                upd1lo = np_.tile([64, nseg], f32, tag="upd1lo")
                nc.sync.dma_start(upd1lo[:], rn01[64:128, :])
                nc.vector.tensor_tensor(out=rn2[:], in0=rn2[:], in1=upd1lo[:],
                                        op=ALU.add)
                nc.vector.tensor_tensor(out=rn2[:], in0=rn2[:], in1=rn01[0:64, :],
                                        op=ALU.add)
                return rn2

            # ---------------- message passing layers ----------------
            for l in range(L):
                dn01 = np_.tile([128, N], f32, tag="dn01")
                rn01 = np_.tile([128, N], f32, tag="rn01")
                dn2 = np_.tile([64, N], f32, tag="dn2")
                rn2 = np_.tile([64, N], f32, tag="rn2")
                fea2 = fp.tile([128, N], f32, tag="fea")
                state = {}
                emit_front(0, fea, w1s[l], b1s[l], state, SBE, 128)
                emit_vs(0, state, SBE)
                emit_front(1, fea, w1s[l], b1s[l], state, SBE, 128)
                for sbi in range(SB):
                    emit_back(sbi, state, re5, wmss[l], wgss[l],
                              wlins[l], rids[l], dn01, rn01, dn2, rn2,
                              SBE, 128, fin=(fea, fea2))
                    if sbi + 1 < SB:
                        emit_vs(sbi + 1, state, SBE)
                    if sbi + 2 < SB:
                        emit_front(sbi + 2, fea, w1s[l], b1s[l], state,
                                   SBE, 128)
                fea = fea2

            # ---------------- crystal pooling ----------------
            dn01 = np_.tile([128, GPC], f32, tag="dn01")
            rn01 = np_.tile([128, GPC], f32, tag="rn01")
            dn2 = np_.tile([64, GPC], f32, tag="dn2")
            rn2 = np_.tile([64, GPC], f32, tag="rn2")
            state = {}
            emit_front(0, fea, cw1, cb1, state, 512, 64)
            emit_vs(0, state, 512)
            emit_back(0, state, rc5, cwms, cwgs, cwlin, crid,
                      dn01, rn01, dn2, rn2, 512, 64)
            cry = finish_update(dn01, rn01, dn2, rn2, GPC)
            nc.sync.dma_start(d_out[:], cry[:])

    nc.compile()
    return nc


def _prep_core_inputs(core, elem_weights, elem_fea_in, W_init, b_init,
                      mg_W1, mg_b1, mg_W2, mg_b2, mm_W1, mm_b1, mm_W2, mm_b2,
                      m_pow, cg_W1, cg_b1, cg_W2, cg_b2, cm_W1, cm_b1, cm_W2,
                      cm_b2, c_pow):
    import ml_dtypes
    f = np.float32
    bf = ml_dtypes.bfloat16
    n0 = core * N
    w = np.ascontiguousarray(elem_weights[n0:n0 + N]).astype(f)
    ef = np.ascontiguousarray(elem_fea_in[n0:n0 + N]).astype(f)

    ins = {}
    ins["eft"] = np.ascontiguousarray(ef.T).astype(bf)
    wi = np.zeros((128, 126), f)
    wi[0:128, 0:63] = W_init[0:128]
    wi[0:72, 63:126] = W_init[128:200]
    ins["wipack"] = wi.astype(bf)
    ins["binit"] = b_init.reshape(63, 1).astype(f)
    ins["wrow"] = w.reshape(1, N)

    def hilo(x64):
        hi = x64.astype(bf).astype(np.float64)
        lo = (x64 - hi).astype(bf)
        return hi.astype(bf), lo

    # edge rider rows: [hiE, loE, hiE, maskE, onesE]
    j_of_e = np.tile(np.arange(K), GPC * K)
    gi_of_e = np.repeat(np.arange(GPC * K), K)
    g_of_e = gi_of_e // K
    i_of_e = gi_of_e % K
    wn = w[g_of_e * K + j_of_e].astype(np.float64)
    hiE, loE = hilo(np.log(wn))
    maskE = np.where(i_of_e == j_of_e, MASKNEG, 0.0)
    re5 = np.zeros((5, E), bf)
    re5[0] = hiE; re5[1] = loE; re5[2] = hiE
    re5[3] = maskE.astype(bf); re5[4] = 1.0
    ins["riderE5"] = re5
    hiC, loC = hilo(np.log(w.astype(np.float64)))
    rc5 = np.zeros((5, N), bf)
    rc5[0] = hiC; rc5[1] = loC; rc5[2] = hiC
    rc5[3] = 0.0; rc5[4] = 1.0
    ins["riderC5"] = rc5

    def pack_wap(W1g, b1g, W2g, b2g, W1m, b1m, W2m, b2m, pw, indim):
        """W1g/W1m: [H, indim(2F or F), HID]; W2g: [H,HID]; W2m: [H,HID,F];
        b2m: [H,F]; b2g: [H]; pw: [H]. Returns the packed tensors."""
        w1 = np.zeros((128, 1536), f)
        b1 = np.zeros((128, 24), f)
        wms = np.zeros((128, 768), f)
        wgs = np.zeros((128, 768), f)
        wlin = np.zeros((128, 384), f)
        rid = np.zeros((5, 384), np.float64)
        for h in range(H):
            for mlp, (W1x, b1x) in enumerate(((W1g[h], b1g[h]), (W1m[h], b1m[h]))):
                for half in range(2):
                    t = h * 4 + mlp * 2 + half
                    w1[0:indim, t * 128:(t + 1) * 128] = \
                        W1x[:, half * 128:(half + 1) * 128]
                    b1[:, t] = b1x[half * 128:(half + 1) * 128]
                    if t >= NDM:
                        b1[:, 12 + (t - NDM)] = -b1x[half * 128:(half + 1) * 128]
            for k in range(4):
                half = k % 2
                hsl = slice(half * 128, (half + 1) * 128)
                col = slice(h * 256 + k * 64, h * 256 + k * 64 + 64)
                tg = h * 4 + 0 * 2 + half
                tm = h * 4 + 1 * 2 + half
                if k < 2:
                    wms[:, col] = (LAM * ALPHA / H) * W2m[h][hsl]
                    wgs[:, col] = np.repeat(
                        ((LAM * ALPHA) * W2g[h][hsl])[:, None], 64, 1)
                else:
                    sgm = -1.0 if tm < NDM else 1.0
                    sgg = -1.0 if tg < NDM else 1.0
                    wms[:, col] = sgm * (LAM / H) * W2m[h][hsl]
                    wgs[:, col] = np.repeat(
                        (sgg * LAM * W2g[h][hsl])[:, None], 64, 1)
            # linear path + consts
            glin = LAM * (W1g[h] @ W2g[h])                       # [indim]
            mlin = (LAM / H) * (W1m[h] @ W2m[h])                 # [indim, F]
            gconst = b2g[h] - LAM * ALPHA * W2g[h].sum() + LAM * (b1g[h] @ W2g[h])
            mconst = (b2m[h] - LAM * ALPHA * W2m[h].sum(0)
                      + LAM * (b1m[h] @ W2m[h])) / H             # [F]
            pw_hi = np.float64(np.float32(bf(pw[h])))
            pw_lo = np.float64(pw[h]) - pw_hi
            if h < 2:
                cols = slice(h * 64, (h + 1) * 64)
                wlin[0:indim, cols] = np.repeat(glin[:, None], 64, 1)
                wlin[0:indim, 128 + h * 64:128 + (h + 1) * 64] = mlin
                rid[0, cols] = pw_hi; rid[1, cols] = pw_hi
                rid[2, cols] = pw_lo; rid[3, cols] = 1.0
                rid[4, cols] = gconst
                rid[4, 128 + h * 64:128 + (h + 1) * 64] = mconst
            else:
                wlin[0:indim, 256:320] = mlin
                wlin[0:indim, 320:384] = np.repeat(glin[:, None], 64, 1)
                rid[4, 256:320] = mconst
                rid[0, 320:384] = pw_hi; rid[1, 320:384] = pw_hi
                rid[2, 320:384] = pw_lo; rid[3, 320:384] = 1.0
                rid[4, 320:384] = gconst
        return (w1.astype(bf), b1, wms.astype(bf), wgs.astype(bf),
                wlin.astype(bf), rid.astype(f).astype(bf))

    for l in range(L):
        w1, b1, wms, wgs, wlin, rid = pack_wap(
            mg_W1[l], mg_b1[l], mg_W2[l], mg_b2[l],
            mm_W1[l], mm_b1[l], mm_W2[l], mm_b2[l], m_pow[l], 2 * F)
        ins[f"w1pack{l}"] = w1
        ins[f"b1pack{l}"] = b1
        ins[f"wms{l}"] = wms
        ins[f"wgs{l}"] = wgs
        ins[f"wlin{l}"] = wlin
        ins[f"rid{l}"] = rid

    w1, b1, wms, wgs, wlin, rid = pack_wap(
        cg_W1, cg_b1, cg_W2, cg_b2, cm_W1, cm_b1, cm_W2, cm_b2, c_pow, F)
    ins["cw1pack"] = w1
    ins["cb1pack"] = b1
    ins["cwms"] = wms
    ins["cwgs"] = wgs
    ins["cwlin"] = wlin
    ins["crid"] = rid
    return {k: np.ascontiguousarray(v) for k, v in ins.items()}


def _check_structure(batch, self_idx, nbr_idx):
    exp_batch = np.repeat(np.arange(G, dtype=np.int64), K)
    i = np.arange(K)
    src, dst = np.meshgrid(i, i, indexing="ij")
    m = src != dst
    offs = (np.arange(G) * K)[:, None]
    exp_self = (offs + src[m][None, :]).reshape(-1)
    exp_nbr = (offs + dst[m][None, :]).reshape(-1)
    if not (np.array_equal(np.asarray(batch, np.int64), exp_batch)
            and np.array_equal(np.asarray(self_idx, np.int64), exp_self)
            and np.array_equal(np.asarray(nbr_idx, np.int64), exp_nbr)):
        raise NotImplementedError(
            "kernel specialized to the 256x16 fully-connected mesh structure")


def kernel(**inputs):
    from concourse.bass_utils import run_bass_kernel_spmd

    _check_structure(inputs["batch"], inputs["self_idx"], inputs["nbr_idx"])
    args = {k: np.asarray(v) for k, v in inputs.items()
            if k not in ("batch", "self_idx", "nbr_idx")}

    if "nc" not in _PROGRAM_CACHE:
        _PROGRAM_CACHE["nc"] = _build_program()
    nc = _PROGRAM_CACHE["nc"]

    in_maps = [_prep_core_inputs(c, **args) for c in range(NCORES)]
    res = run_bass_kernel_spmd(nc, in_maps, list(range(NCORES)))
    out = np.concatenate([res.results[c]["out"].T for c in range(NCORES)], axis=0)
    return out.astype(np.float32)


# revision 12
# speedup vs baseline: 1.0061x; 1.0061x over previous
"""Trainium2 Bass kernel for the Roost-style GNN (nn_DescriptorNetworkTorch).

Data-parallel over graphs: 256 graphs of 16 fully-connected atoms sharded as
32 graphs per NeuronCore across 8 cores; params replicated; no collectives.

v2 restructure (vs the exp/min/relu baseline):
  selu(y) = LAM*ALPHA*(e^m - 1) + LAM*(y - m)  with  m = min(y, 0)
  - m computed in ONE DVE pass from PSUM (or, for 3 of 12 families, as
    m' = relu(-y) on the Scalar engine to balance engine load)
  - v = e^m via wide-batched Scalar ACTIVATE over an SBUF super-tile
  - the linear  LAM*W2*y  term folds into a precomputed (W1@W2) matmul on
    the PE (reading the pair tile), so no relu tiles exist at all
  - W2 contracts stacked [v; m] (K=512) + linear + a K=5 rider that carries
    pow*ln(w) (hi/lo bf16 split), the softmax diagonal mask, and all scalar
    constants (b2, -LAM*ALPHA*sum(W2), LAM*b1@W2), assembled on host
  - edges processed in superblocks of 1024 (4 graphs) with 2-bank PSUM
    tiles so every elementwise instruction runs at 1024 free-size
  - softmax: z = exp(gate) in bf16, segmented reduces on DVE
  - pair tiles built by the (otherwise idle) GpSimd engine
"""

import numpy as np

G, K, F, EMB, HID, L, H = 256, 16, 64, 200, 256, 3, 3
NCORES = 8
GPC = G // NCORES          # graphs per core (32)
N = GPC * K                # nodes per core (512)
E = GPC * K * K            # all-pair edges per core (8192)
SB = 8                     # superblocks per layer (1024 edges each)
SBE = 1024
NT = 12                    # hid families: t = h*4 + mlp*2 + half
NDM = 9                    # families 0..8: m on DVE; 9..11: m' = relu(-y) on Scalar
LAM = 1.0507009873554804934193349852946
ALPHA = 1.6732632423543772848170429916717
MASKNEG = -1e30

_PROGRAM_CACHE = {}


def _build_program():
    import concourse.bass as bass
    import concourse.bacc as bacc
    import concourse.mybir as mybir
    import concourse.tile as tile

    dt = mybir.dt
    AF = mybir.ActivationFunctionType
    ALU = mybir.AluOpType
    AX = mybir.AxisListType
    f32 = dt.float32
    bf16 = dt.bfloat16

    nc = bacc.Bacc("TRN2", target_bir_lowering=False, debug=False,
                   num_devices=NCORES)

    # ---------------- DRAM I/O ----------------
    d_eft = nc.dram_tensor("eft", [EMB, N], bf16, kind="ExternalInput")
    d_wi = nc.dram_tensor("wipack", [128, 126], bf16, kind="ExternalInput")
    d_binit = nc.dram_tensor("binit", [63, 1], f32, kind="ExternalInput")
    d_wrow = nc.dram_tensor("wrow", [1, N], f32, kind="ExternalInput")
    d_re5 = nc.dram_tensor("riderE5", [5, E], bf16, kind="ExternalInput")
    d_rc5 = nc.dram_tensor("riderC5", [5, N], bf16, kind="ExternalInput")
    d_w1, d_b1, d_wms, d_wgs, d_wlin, d_rid = [], [], [], [], [], []
    for l in range(L):
        d_w1.append(nc.dram_tensor(f"w1pack{l}", [128, 1536], bf16, kind="ExternalInput"))
        d_b1.append(nc.dram_tensor(f"b1pack{l}", [128, 24], f32, kind="ExternalInput"))
        d_wms.append(nc.dram_tensor(f"wms{l}", [128, 768], bf16, kind="ExternalInput"))
        d_wgs.append(nc.dram_tensor(f"wgs{l}", [128, 768], bf16, kind="ExternalInput"))
        d_wlin.append(nc.dram_tensor(f"wlin{l}", [128, 384], bf16, kind="ExternalInput"))
        d_rid.append(nc.dram_tensor(f"rid{l}", [5, 384], bf16, kind="ExternalInput"))
    d_cw1 = nc.dram_tensor("cw1pack", [128, 1536], bf16, kind="ExternalInput")
    d_cb1 = nc.dram_tensor("cb1pack", [128, 24], f32, kind="ExternalInput")
    d_cwms = nc.dram_tensor("cwms", [128, 768], bf16, kind="ExternalInput")
    d_cwgs = nc.dram_tensor("cwgs", [128, 768], bf16, kind="ExternalInput")
    d_cwlin = nc.dram_tensor("cwlin", [128, 384], bf16, kind="ExternalInput")
    d_crid = nc.dram_tensor("crid", [5, 384], bf16, kind="ExternalInput")
    d_out = nc.dram_tensor("out", [F, GPC], f32, kind="ExternalOutput")

    with tile.TileContext(nc) as tc:
        with tc.tile_pool(name="const", bufs=1) as cp, \
             tc.tile_pool(name="fea", bufs=2) as fp, \
             tc.tile_pool(name="sup", bufs=2) as sp, \
             tc.tile_pool(name="zp", bufs=2) as zp, \
             tc.tile_pool(name="node", bufs=1) as np_, \
             tc.tile_pool(name="pre_ps", bufs=2, space="PSUM") as pps, \
             tc.tile_pool(name="w2_ps", bufs=2, space="PSUM") as wps:

            # ---- load constants ----
            ef1 = cp.tile([128, N], bf16, tag="ef1")
            ef2 = cp.tile([72, N], bf16, tag="ef2")
            nc.scalar.dma_start(ef1[0:64, :], d_eft[0:64, :])
            nc.sync.dma_start(ef1[64:128, :], d_eft[64:128, :])
            nc.gpsimd.dma_start(ef2[:], d_eft[128:200, :])
            wi = cp.tile([128, 126], bf16, tag="wi")
            nc.sync.dma_start(wi[:], d_wi[:])
            binit = cp.tile([63, 1], f32, tag="binit")
            nc.sync.dma_start(binit[:], d_binit[:])
            wrow = cp.tile([1, N], f32, tag="wrow")
            nc.sync.dma_start(wrow[:], d_wrow[:])
            w1s, b1s, wmss, wgss, wlins, rids = [], [], [], [], [], []
            for l in range(L):
                t = cp.tile([128, 1536], bf16, tag=f"w1_{l}")
                if l == 0:
                    nc.gpsimd.dma_start(t[:, 0:768], d_w1[l][:, 0:768])
                    nc.scalar.dma_start(t[:, 768:1536], d_w1[l][:, 768:1536])
                else:
                    nc.sync.dma_start(t[:], d_w1[l][:])
                w1s.append(t)
                t = cp.tile([128, 24], f32, tag=f"b1_{l}")
                nc.sync.dma_start(t[:], d_b1[l][:]); b1s.append(t)
                t = cp.tile([128, 768], bf16, tag=f"wms_{l}")
                (nc.gpsimd if l == 0 else nc.sync).dma_start(t[:], d_wms[l][:]); wmss.append(t)
                t = cp.tile([128, 768], bf16, tag=f"wgs_{l}")
                (nc.scalar if l == 0 else nc.sync).dma_start(t[:], d_wgs[l][:]); wgss.append(t)
                t = cp.tile([128, 384], bf16, tag=f"wlin_{l}")
                nc.sync.dma_start(t[:], d_wlin[l][:]); wlins.append(t)
                t = cp.tile([5, 384], bf16, tag=f"rid_{l}")
                nc.sync.dma_start(t[:], d_rid[l][:]); rids.append(t)
            re5 = cp.tile([5, E], bf16, tag="re5")
            nc.sync.dma_start(re5[:], d_re5[:])
            rc5 = cp.tile([5, N], bf16, tag="rc5")
            nc.sync.dma_start(rc5[:], d_rc5[:])
            cw1 = cp.tile([128, 1536], bf16, tag="cw1")
            nc.sync.dma_start(cw1[:], d_cw1[:])
            cb1 = cp.tile([128, 24], f32, tag="cb1")
            nc.sync.dma_start(cb1[:], d_cb1[:])
            cwms = cp.tile([128, 768], bf16, tag="cwms")
            nc.sync.dma_start(cwms[:], d_cwms[:])
            cwgs = cp.tile([128, 768], bf16, tag="cwgs")
            nc.sync.dma_start(cwgs[:], d_cwgs[:])
            cwlin = cp.tile([128, 384], bf16, tag="cwlin")
            nc.sync.dma_start(cwlin[:], d_cwlin[:])
            crid = cp.tile([5, 384], bf16, tag="crid")
            nc.sync.dma_start(crid[:], d_crid[:])

            # ---- initial embed ----
            fea = fp.tile([128, N], f32, tag="fea")
            for c in range(N // 512):
                sl = slice(c * 512, (c + 1) * 512)
                emb_ps = wps.tile([63, 512], f32, tag="w2")
                nc.tensor.matmul(emb_ps[:], (wi[0:128, 0:63]), (ef1[:, sl]),
                                 start=True, stop=False)
                nc.tensor.matmul(emb_ps[:], (wi[0:72, 63:126]), (ef2[:, sl]),
                                 start=False, stop=True)
                nc.scalar.activation(fea[0:63, sl], emb_ps[:], AF.Identity,
                                     bias=binit[:], scale=1.0)
            nc.sync.dma_start(fea[63:64, :], wrow[:])
            nc.sync.dma_start(fea[64:128, :], fea[0:64, :])

            U_OF_T = {0: 0, 1: 1, 4: 2, 5: 3, 2: 4, 3: 5, 6: 6, 7: 7,
                      8: 8, 9: 9, 10: 10, 11: 11}

            def mslice(t, nh, sbe):
                return ("M", U_OF_T[t] * sbe + nh * 512)

            def emit_front(sbi, fea_t, w1t, b1t, state, sbe, kdim):
                """pair build + W1 matmuls + m/m' for superblock sbi."""
                pair = sp.tile([128, sbe], bf16, tag="pair", bufs=3)
                if kdim == 128:
                    colb = sbi * (sbe // K)
                    ng = sbe // 256
                    self_src = (fea_t[0:64, colb:colb + ng * K]
                                .rearrange("p (g i) -> p g i", g=ng)
                                .unsqueeze(3).broadcast_to([64, ng, K, K]))
                    nbr_src = (fea_t[64:128, colb:colb + ng * K]
                               .rearrange("p (g j) -> p g j", g=ng)
                               .unsqueeze(2).broadcast_to([64, ng, K, K]))
                    nc.vector.tensor_copy(
                        pair[0:64, :].rearrange("p (g i j) -> p g i j", g=ng, i=K),
                        self_src)
                    nc.scalar.activation(
                        pair[64:128, :].rearrange("p (g i j) -> p g i j", g=ng, i=K),
                        nbr_src, AF.Identity)
                else:
                    nc.vector.tensor_copy(pair[0:64, :], fea_t[0:64, :])
                mAB = sp.tile([128, NT * sbe], bf16, tag="mAB", bufs=3)
                for t in sorted(range(NT), key=lambda t: U_OF_T[t]):
                    u = U_OF_T[t]
                    pre = pps.tile([128, sbe], f32, tag="pre")
                    for nh in range(sbe // 512):
                        nc.tensor.matmul(pre[:, nh * 512:nh * 512 + 512],
                                         (w1t[0:kdim, t * 128:t * 128 + 128]),
                                         (pair[0:kdim, nh * 512:nh * 512 + 512]),
                                         start=True, stop=True)
                    if t < NDM:
                        nc.vector.tensor_scalar(
                            mAB[:, u * sbe:(u + 1) * sbe], pre[:],
                            b1t[:, t:t + 1], 0.0, op0=ALU.add, op1=ALU.min)
                    else:
                        nc.scalar.activation(
                            mAB[:, u * sbe:(u + 1) * sbe], pre[:],
                            AF.Relu, bias=b1t[:, 12 + (t - NDM):13 + (t - NDM)],
                            scale=-1.0)
                state[sbi] = (pair, mAB, None)

            def emit_vs(sbi, state, sbe):
                """wide-batched v = e^m for superblock sbi."""
                pair, mAB, _ = state[sbi]
                vS = sp.tile([128, NT * sbe], bf16, tag="vS", bufs=2)
                nc.scalar.activation(vS[:, 0:4 * sbe], mAB[:, 0:4 * sbe],
                                     AF.Exp)
                nc.scalar.activation(vS[:, 4 * sbe:8 * sbe],
                                     mAB[:, 4 * sbe:8 * sbe], AF.Exp)
                nc.scalar.activation(vS[:, 8 * sbe:9 * sbe],
                                     mAB[:, 8 * sbe:9 * sbe], AF.Exp)
                nc.scalar.activation(vS[:, 9 * sbe:12 * sbe],
                                     mAB[:, 9 * sbe:12 * sbe], AF.Exp,
                                     scale=-1.0)
                state[sbi] = (pair, mAB, vS)

            def emit_back(sbi, state, rider, wmst, wgst, wlint, ridt,
                          dn01, rn01, dn2, rn2, sbe, kdim, fin=None):
                """W2 matmuls + softmax + segmented reduces for superblock sbi."""
                pair, mAB, vS = state.pop(sbi)
                nhs = sbe // 512

                def rhs(kind, off):
                    src = mAB if kind == "M" else vS
                    return src[:, off:off + 512]

                def stacked(ps, h, mlp, base, tp):
                    """8 accumulating chunk matmuls for head h of mlp into
                    row-block tp of ps (per nh bank)."""
                    rows = slice(64, 128) if tp else slice(0, 64)
                    wt = wgst if mlp == 0 else wmst
                    for nh in range(nhs):
                        for k in range(4):
                            half = k % 2
                            t = h * 4 + mlp * 2 + half
                            if k < 2:
                                kind, off = "V", U_OF_T[t] * sbe + nh * 512
                            else:
                                kind, off = mslice(t, nh, sbe)
                            last = (k == 3)
                            nc.tensor.matmul(
                                ps[rows, nh * 512:nh * 512 + 512],
                                (wt[:, h * 256 + k * 64:h * 256 + k * 64 + 64]),
                                (rhs(kind, off)),
                                start=False, stop=(last and base),
                                tile_position=((0, 64) if tp else None),
                                skip_group_check=True)

                def headpair(rcol, lcol, hs, mlps, esl0):
                    """one W2 PSUM tile: rider + linear + 2 row-blocks."""
                    ps = wps.tile([128, sbe], f32, tag="w2")
                    for nh in range(nhs):
                        esl = slice(esl0 + sbi * sbe + nh * 512,
                                    esl0 + sbi * sbe + nh * 512 + 512)
                        nc.tensor.matmul(ps[:, nh * 512:nh * 512 + 512],
                                         (ridt[0:5, rcol:rcol + 128]),
                                         (rider[:, esl]),
                                         start=True, stop=False,
                                         skip_group_check=True)
                        nc.tensor.matmul(ps[:, nh * 512:nh * 512 + 512],
                                         (wlint[0:kdim, lcol:lcol + 128]),
                                         (pair[0:kdim, nh * 512:nh * 512 + 512]),
                                         start=False, stop=False,
                                         skip_group_check=True)
                    stacked(ps, hs[0], mlps[0], False, False)
                    stacked(ps, hs[1], mlps[1], True, True)
                    return ps

                gate01 = headpair(0, 0, (0, 1), (0, 0), 0)
                msg01 = headpair(128, 128, (0, 1), (1, 1), 0)
                g2m2 = headpair(256, 256, (2, 2), (1, 0), 0)

                nseg = sbe // K
                seg = slice(sbi * nseg, (sbi + 1) * nseg)
                z01 = zp.tile([128, sbe], bf16, tag="z01")
                nc.scalar.activation(z01[:], gate01[:], AF.Exp)
                nc.vector.tensor_reduce(
                    out=dn01[:, seg],
                    in_=z01[:].rearrange("p (s j) -> p s j", j=K),
                    axis=AX.X, op=ALU.add)
                prod01 = zp.tile([128, sbe], bf16, tag="prod01")
                nc.vector.tensor_tensor(out=prod01[:], in0=msg01[:], in1=z01[:],
                                        op=ALU.mult)
                nc.vector.tensor_reduce(
                    out=rn01[:, seg],
                    in_=prod01[:].rearrange("p (s j) -> p s j", j=K),
                    axis=AX.X, op=ALU.add)
                z2 = zp.tile([64, sbe], bf16, tag="z2")
                nc.scalar.activation(z2[:], g2m2[64:128, :], AF.Exp)
                nc.vector.tensor_reduce(
                    out=dn2[:, seg],
                    in_=z2[:].rearrange("p (s j) -> p s j", j=K),
                    axis=AX.X, op=ALU.add)
                prod2 = zp.tile([64, sbe], bf16, tag="prod2")
                nc.vector.tensor_tensor(out=prod2[:], in0=g2m2[0:64, :],
                                        in1=z2[:], op=ALU.mult)
                nc.vector.tensor_reduce(
                    out=rn2[:, seg],
                    in_=prod2[:].rearrange("p (s j) -> p s j", j=K),
                    axis=AX.X, op=ALU.add)
                if fin is not None:
                    fea_src, fea_dst = fin
                    nc.vector.reciprocal(dn01[:, seg], dn01[:, seg])
                    nc.vector.reciprocal(dn2[:, seg], dn2[:, seg])
                    nc.vector.tensor_tensor(out=rn01[:, seg], in0=rn01[:, seg],
                                            in1=dn01[:, seg], op=ALU.mult)
                    nc.vector.tensor_tensor(out=rn2[:, seg], in0=rn2[:, seg],
                                            in1=dn2[:, seg], op=ALU.mult)
                    u1lo = np_.tile([64, nseg], f32, tag="u1lo", bufs=3)
                    nc.sync.dma_start(u1lo[:], rn01[64:128, seg])
                    nc.gpsimd.tensor_tensor(out=rn2[:, seg], in0=rn2[:, seg],
                                            in1=u1lo[:], op=ALU.add)
                    nc.gpsimd.tensor_tensor(out=rn2[:, seg], in0=rn2[:, seg],
                                            in1=rn01[0:64, seg], op=ALU.add)
                    nc.gpsimd.tensor_tensor(out=fea_dst[0:64, seg],
                                            in0=fea_src[0:64, seg],
                                            in1=rn2[:, seg], op=ALU.add)
                    nc.sync.dma_start(fea_dst[64:128, seg], fea_dst[0:64, seg])

            def finish_update(dn01, rn01, dn2, rn2, nseg):
                nc.vector.reciprocal(dn01[:], dn01[:])
                nc.vector.reciprocal(dn2[:], dn2[:])
                nc.vector.tensor_tensor(out=rn01[:], in0=rn01[:], in1=dn01[:],
                                        op=ALU.mult)
                nc.vector.tensor_tensor(out=rn2[:], in0=rn2[:], in1=dn2[:],
                                        op=ALU.mult)
                upd1lo = np_.tile([64, nseg], f32, tag="upd1lo")
                nc.sync.dma_start(upd1lo[:], rn01[64:128, :])
                nc.vector.tensor_tensor(out=rn2[:], in0=rn2[:], in1=upd1lo[:],
                                        op=ALU.add)
                nc.vector.tensor_tensor(out=rn2[:], in0=rn2[:], in1=rn01[0:64, :],
                                        op=ALU.add)
                return rn2

            # ---------------- message passing layers ----------------
            for l in range(L):
                dn01 = np_.tile([128, N], f32, tag="dn01")
                rn01 = np_.tile([128, N], f32, tag="rn01")
                dn2 = np_.tile([64, N], f32, tag="dn2")
                rn2 = np_.tile([64, N], f32, tag="rn2")
                fea2 = fp.tile([128, N], f32, tag="fea")
                state = {}
                emit_front(0, fea, w1s[l], b1s[l], state, SBE, 128)
                emit_vs(0, state, SBE)
                emit_front(1, fea, w1s[l], b1s[l], state, SBE, 128)
                for sbi in range(SB):
                    emit_back(sbi, state, re5, wmss[l], wgss[l],
                              wlins[l], rids[l], dn01, rn01, dn2, rn2,
                              SBE, 128, fin=(fea, fea2))
                    if sbi + 1 < SB:
                        emit_vs(sbi + 1, state, SBE)
                    if sbi + 2 < SB:
                        emit_front(sbi + 2, fea, w1s[l], b1s[l], state,
                                   SBE, 128)
                fea = fea2

            # ---------------- crystal pooling ----------------
            dn01 = np_.tile([128, GPC], f32, tag="dn01")
            rn01 = np_.tile([128, GPC], f32, tag="rn01")
            dn2 = np_.tile([64, GPC], f32, tag="dn2")
            rn2 = np_.tile([64, GPC], f32, tag="rn2")
            state = {}
            emit_front(0, fea, cw1, cb1, state, 512, 64)
            emit_vs(0, state, 512)
            emit_back(0, state, rc5, cwms, cwgs, cwlin, crid,
                      dn01, rn01, dn2, rn2, 512, 64)
            cry = finish_update(dn01, rn01, dn2, rn2, GPC)
            nc.sync.dma_start(d_out[:], cry[:])

    nc.compile()
    return nc


def _prep_core_inputs(core, elem_weights, elem_fea_in, W_init, b_init,
                      mg_W1, mg_b1, mg_W2, mg_b2, mm_W1, mm_b1, mm_W2, mm_b2,
                      m_pow, cg_W1, cg_b1, cg_W2, cg_b2, cm_W1, cm_b1, cm_W2,
                      cm_b2, c_pow):
    import ml_dtypes
    f = np.float32
    bf = ml_dtypes.bfloat16
    n0 = core * N
    w = np.ascontiguousarray(elem_weights[n0:n0 + N]).astype(f)
    ef = np.ascontiguousarray(elem_fea_in[n0:n0 + N]).astype(f)

    ins = {}
    ins["eft"] = np.ascontiguousarray(ef.T).astype(bf)
    wi = np.zeros((128, 126), f)
    wi[0:128, 0:63] = W_init[0:128]
    wi[0:72, 63:126] = W_init[128:200]
    ins["wipack"] = wi.astype(bf)
    ins["binit"] = b_init.reshape(63, 1).astype(f)
    ins["wrow"] = w.reshape(1, N)

    def hilo(x64):
        hi = x64.astype(bf).astype(np.float64)
        lo = (x64 - hi).astype(bf)
        return hi.astype(bf), lo

    # edge rider rows: [hiE, loE, hiE, maskE, onesE]
    j_of_e = np.tile(np.arange(K), GPC * K)
    gi_of_e = np.repeat(np.arange(GPC * K), K)
    g_of_e = gi_of_e // K
    i_of_e = gi_of_e % K
    wn = w[g_of_e * K + j_of_e].astype(np.float64)
    hiE, loE = hilo(np.log(wn))
    maskE = np.where(i_of_e == j_of_e, MASKNEG, 0.0)
    re5 = np.zeros((5, E), bf)
    re5[0] = hiE; re5[1] = loE; re5[2] = hiE
    re5[3] = maskE.astype(bf); re5[4] = 1.0
    ins["riderE5"] = re5
    hiC, loC = hilo(np.log(w.astype(np.float64)))
    rc5 = np.zeros((5, N), bf)
    rc5[0] = hiC; rc5[1] = loC; rc5[2] = hiC
    rc5[3] = 0.0; rc5[4] = 1.0
    ins["riderC5"] = rc5

    def pack_wap(W1g, b1g, W2g, b2g, W1m, b1m, W2m, b2m, pw, indim):
        """W1g/W1m: [H, indim(2F or F), HID]; W2g: [H,HID]; W2m: [H,HID,F];
        b2m: [H,F]; b2g: [H]; pw: [H]. Returns the packed tensors."""
        w1 = np.zeros((128, 1536), f)
        b1 = np.zeros((128, 24), f)
        wms = np.zeros((128, 768), f)
        wgs = np.zeros((128, 768), f)
        wlin = np.zeros((128, 384), f)
        rid = np.zeros((5, 384), np.float64)
        for h in range(H):
            for mlp, (W1x, b1x) in enumerate(((W1g[h], b1g[h]), (W1m[h], b1m[h]))):
                for half in range(2):
                    t = h * 4 + mlp * 2 + half
                    w1[0:indim, t * 128:(t + 1) * 128] = \
                        W1x[:, half * 128:(half + 1) * 128]
                    b1[:, t] = b1x[half * 128:(half + 1) * 128]
                    if t >= NDM:
                        b1[:, 12 + (t - NDM)] = -b1x[half * 128:(half + 1) * 128]
            for k in range(4):
                half = k % 2
                hsl = slice(half * 128, (half + 1) * 128)
                col = slice(h * 256 + k * 64, h * 256 + k * 64 + 64)
                tg = h * 4 + 0 * 2 + half
                tm = h * 4 + 1 * 2 + half
                if k < 2:
                    wms[:, col] = (LAM * ALPHA / H) * W2m[h][hsl]
                    wgs[:, col] = np.repeat(
                        ((LAM * ALPHA) * W2g[h][hsl])[:, None], 64, 1)
                else:
                    sgm = -1.0 if tm < NDM else 1.0
                    sgg = -1.0 if tg < NDM else 1.0
                    wms[:, col] = sgm * (LAM / H) * W2m[h][hsl]
                    wgs[:, col] = np.repeat(
                        (sgg * LAM * W2g[h][hsl])[:, None], 64, 1)
            # linear path + consts
            glin = LAM * (W1g[h] @ W2g[h])                       # [indim]
            mlin = (LAM / H) * (W1m[h] @ W2m[h])                 # [indim, F]
            gconst = b2g[h] - LAM * ALPHA * W2g[h].sum() + LAM * (b1g[h] @ W2g[h])
            mconst = (b2m[h] - LAM * ALPHA * W2m[h].sum(0)
                      + LAM * (b1m[h] @ W2m[h])) / H             # [F]
            pw_hi = np.float64(np.float32(bf(pw[h])))
            pw_lo = np.float64(pw[h]) - pw_hi
            if h < 2:
                cols = slice(h * 64, (h + 1) * 64)
                wlin[0:indim, cols] = np.repeat(glin[:, None], 64, 1)
                wlin[0:indim, 128 + h * 64:128 + (h + 1) * 64] = mlin
                rid[0, cols] = pw_hi; rid[1, cols] = pw_hi
                rid[2, cols] = pw_lo; rid[3, cols] = 1.0
                rid[4, cols] = gconst
                rid[4, 128 + h * 64:128 + (h + 1) * 64] = mconst
            else:
                wlin[0:indim, 256:320] = mlin
                wlin[0:indim, 320:384] = np.repeat(glin[:, None], 64, 1)
                rid[4, 256:320] = mconst
                rid[0, 320:384] = pw_hi; rid[1, 320:384] = pw_hi
                rid[2, 320:384] = pw_lo; rid[3, 320:384] = 1.0
                rid[4, 320:384] = gconst
        return (w1.astype(bf), b1, wms.astype(bf), wgs.astype(bf),
                wlin.astype(bf), rid.astype(f).astype(bf))

    for l in range(L):
        w1, b1, wms, wgs, wlin, rid = pack_wap(
            mg_W1[l], mg_b1[l], mg_W2[l], mg_b2[l],
            mm_W1[l], mm_b1[l], mm_W2[l], mm_b2[l], m_pow[l], 2 * F)
        ins[f"w1pack{l}"] = w1
        ins[f"b1pack{l}"] = b1
        ins[f"wms{l}"] = wms
        ins[f"wgs{l}"] = wgs
        ins[f"wlin{l}"] = wlin
        ins[f"rid{l}"] = rid

    w1, b1, wms, wgs, wlin, rid = pack_wap(
        cg_W1, cg_b1, cg_W2, cg_b2, cm_W1, cm_b1, cm_W2, cm_b2, c_pow, F)
    ins["cw1pack"] = w1
    ins["cb1pack"] = b1
    ins["cwms"] = wms
    ins["cwgs"] = wgs
    ins["cwlin"] = wlin
    ins["crid"] = rid
    return {k: np.ascontiguousarray(v) for k, v in ins.items()}


def _check_structure(batch, self_idx, nbr_idx):
    exp_batch = np.repeat(np.arange(G, dtype=np.int64), K)
    i = np.arange(K)
    src, dst = np.meshgrid(i, i, indexing="ij")
    m = src != dst
    offs = (np.arange(G) * K)[:, None]
    exp_self = (offs + src[m][None, :]).reshape(-1)
    exp_nbr = (offs + dst[m][None, :]).reshape(-1)
    if not (np.array_equal(np.asarray(batch, np.int64), exp_batch)
            and np.array_equal(np.asarray(self_idx, np.int64), exp_self)
            and np.array_equal(np.asarray(nbr_idx, np.int64), exp_nbr)):
        raise NotImplementedError(
            "kernel specialized to the 256x16 fully-connected mesh structure")


def kernel(**inputs):
    from concourse.bass_utils import run_bass_kernel_spmd

    _check_structure(inputs["batch"], inputs["self_idx"], inputs["nbr_idx"])
    args = {k: np.asarray(v) for k, v in inputs.items()
            if k not in ("batch", "self_idx", "nbr_idx")}

    if "nc" not in _PROGRAM_CACHE:
        _PROGRAM_CACHE["nc"] = _build_program()
    nc = _PROGRAM_CACHE["nc"]

    in_maps = [_prep_core_inputs(c, **args) for c in range(NCORES)]
    res = run_bass_kernel_spmd(nc, in_maps, list(range(NCORES)))
    out = np.concatenate([res.results[c]["out"].T for c in range(NCORES)], axis=0)
    return out.astype(np.float32)


# revision 13
# speedup vs baseline: 1.0075x; 1.0014x over previous
"""Trainium2 Bass kernel for the Roost-style GNN (nn_DescriptorNetworkTorch).

Data-parallel over graphs: 256 graphs of 16 fully-connected atoms sharded as
32 graphs per NeuronCore across 8 cores; params replicated; no collectives.

v2 restructure (vs the exp/min/relu baseline):
  selu(y) = LAM*ALPHA*(e^m - 1) + LAM*(y - m)  with  m = min(y, 0)
  - m computed in ONE DVE pass from PSUM (or, for 3 of 12 families, as
    m' = relu(-y) on the Scalar engine to balance engine load)
  - v = e^m via wide-batched Scalar ACTIVATE over an SBUF super-tile
  - the linear  LAM*W2*y  term folds into a precomputed (W1@W2) matmul on
    the PE (reading the pair tile), so no relu tiles exist at all
  - W2 contracts stacked [v; m] (K=512) + linear + a K=5 rider that carries
    pow*ln(w) (hi/lo bf16 split), the softmax diagonal mask, and all scalar
    constants (b2, -LAM*ALPHA*sum(W2), LAM*b1@W2), assembled on host
  - edges processed in superblocks of 1024 (4 graphs) with 2-bank PSUM
    tiles so every elementwise instruction runs at 1024 free-size
  - softmax: z = exp(gate) in bf16, segmented reduces on DVE
  - two-superblock-deep software pipeline (back(k) | vS(k+1) | front(k+2))
    keeps all engines fed; attention-normalize + residual are emitted per
    superblock slice so consecutive layers overlap with no serial boundary
  - pair tiles built on DVE (self half) + Scalar (nbr half); GpSimd carries
    the head-mean/residual adds; startup weight DMAs spread over 3 queues
"""

import numpy as np

G, K, F, EMB, HID, L, H = 256, 16, 64, 200, 256, 3, 3
NCORES = 8
GPC = G // NCORES          # graphs per core (32)
N = GPC * K                # nodes per core (512)
E = GPC * K * K            # all-pair edges per core (8192)
SB = 8                     # superblocks per layer (1024 edges each)
SBE = 1024
NT = 12                    # hid families: t = h*4 + mlp*2 + half
NDM = 9                    # families 0..8: m on DVE; 9..11: m' = relu(-y) on Scalar
LAM = 1.0507009873554804934193349852946
ALPHA = 1.6732632423543772848170429916717
MASKNEG = -1e30

_PROGRAM_CACHE = {}


def _build_program():
    import concourse.bass as bass
    import concourse.bacc as bacc
    import concourse.mybir as mybir
    import concourse.tile as tile

    dt = mybir.dt
    AF = mybir.ActivationFunctionType
    ALU = mybir.AluOpType
    AX = mybir.AxisListType
    f32 = dt.float32
    bf16 = dt.bfloat16

    nc = bacc.Bacc("TRN2", target_bir_lowering=False, debug=False,
                   num_devices=NCORES)

    # ---------------- DRAM I/O ----------------
    d_eft = nc.dram_tensor("eft", [EMB, N], bf16, kind="ExternalInput")
    d_wi = nc.dram_tensor("wipack", [128, 126], bf16, kind="ExternalInput")
    d_binit = nc.dram_tensor("binit", [63, 1], f32, kind="ExternalInput")
    d_wrow = nc.dram_tensor("wrow", [1, N], f32, kind="ExternalInput")
    d_re5 = nc.dram_tensor("riderE5", [5, E], bf16, kind="ExternalInput")
    d_rc5 = nc.dram_tensor("riderC5", [5, N], bf16, kind="ExternalInput")
    d_w1, d_b1, d_wms, d_wgs, d_wlin, d_rid = [], [], [], [], [], []
    for l in range(L):
        d_w1.append(nc.dram_tensor(f"w1pack{l}", [128, 1536], bf16, kind="ExternalInput"))
        d_b1.append(nc.dram_tensor(f"b1pack{l}", [128, 24], f32, kind="ExternalInput"))
        d_wms.append(nc.dram_tensor(f"wms{l}", [128, 768], bf16, kind="ExternalInput"))
        d_wgs.append(nc.dram_tensor(f"wgs{l}", [128, 768], bf16, kind="ExternalInput"))
        d_wlin.append(nc.dram_tensor(f"wlin{l}", [128, 384], bf16, kind="ExternalInput"))
        d_rid.append(nc.dram_tensor(f"rid{l}", [5, 384], bf16, kind="ExternalInput"))
    d_cw1 = nc.dram_tensor("cw1pack", [128, 1536], bf16, kind="ExternalInput")
    d_cb1 = nc.dram_tensor("cb1pack", [128, 24], f32, kind="ExternalInput")
    d_cwms = nc.dram_tensor("cwms", [128, 768], bf16, kind="ExternalInput")
    d_cwgs = nc.dram_tensor("cwgs", [128, 768], bf16, kind="ExternalInput")
    d_cwlin = nc.dram_tensor("cwlin", [128, 384], bf16, kind="ExternalInput")
    d_crid = nc.dram_tensor("crid", [5, 384], bf16, kind="ExternalInput")
    d_out = nc.dram_tensor("out", [F, GPC], f32, kind="ExternalOutput")

    with tile.TileContext(nc) as tc:
        with tc.tile_pool(name="const", bufs=1) as cp, \
             tc.tile_pool(name="fea", bufs=2) as fp, \
             tc.tile_pool(name="sup", bufs=2) as sp, \
             tc.tile_pool(name="zp", bufs=2) as zp, \
             tc.tile_pool(name="node", bufs=1) as np_, \
             tc.tile_pool(name="pre_ps", bufs=2, space="PSUM") as pps, \
             tc.tile_pool(name="w2_ps", bufs=2, space="PSUM") as wps:

            # ---- load constants ----
            ef1 = cp.tile([128, N], bf16, tag="ef1")
            ef2 = cp.tile([72, N], bf16, tag="ef2")
            nc.scalar.dma_start(ef1[0:64, :], d_eft[0:64, :])
            nc.sync.dma_start(ef1[64:128, :], d_eft[64:128, :])
            nc.gpsimd.dma_start(ef2[:], d_eft[128:200, :])
            wi = cp.tile([128, 126], bf16, tag="wi")
            nc.sync.dma_start(wi[:], d_wi[:])
            binit = cp.tile([63, 1], f32, tag="binit")
            nc.sync.dma_start(binit[:], d_binit[:])
            wrow = cp.tile([1, N], f32, tag="wrow")
            nc.sync.dma_start(wrow[:], d_wrow[:])
            w1s, b1s, wmss, wgss, wlins, rids = [], [], [], [], [], []
            for l in range(L):
                t = cp.tile([128, 1536], bf16, tag=f"w1_{l}")
                if l == 0:
                    nc.gpsimd.dma_start(t[:, 0:768], d_w1[l][:, 0:768])
                    nc.scalar.dma_start(t[:, 768:1536], d_w1[l][:, 768:1536])
                else:
                    nc.sync.dma_start(t[:], d_w1[l][:])
                w1s.append(t)
                t = cp.tile([128, 24], f32, tag=f"b1_{l}")
                nc.sync.dma_start(t[:], d_b1[l][:]); b1s.append(t)
                t = cp.tile([128, 768], bf16, tag=f"wms_{l}")
                (nc.gpsimd if l == 0 else nc.sync).dma_start(t[:], d_wms[l][:]); wmss.append(t)
                t = cp.tile([128, 768], bf16, tag=f"wgs_{l}")
                (nc.scalar if l == 0 else nc.sync).dma_start(t[:], d_wgs[l][:]); wgss.append(t)
                t = cp.tile([128, 384], bf16, tag=f"wlin_{l}")
                nc.sync.dma_start(t[:], d_wlin[l][:]); wlins.append(t)
                t = cp.tile([5, 384], bf16, tag=f"rid_{l}")
                nc.sync.dma_start(t[:], d_rid[l][:]); rids.append(t)
            re5 = cp.tile([5, E], bf16, tag="re5")
            nc.sync.dma_start(re5[:], d_re5[:])
            rc5 = cp.tile([5, N], bf16, tag="rc5")
            nc.sync.dma_start(rc5[:], d_rc5[:])
            cw1 = cp.tile([128, 1536], bf16, tag="cw1")
            nc.sync.dma_start(cw1[:], d_cw1[:])
            cb1 = cp.tile([128, 24], f32, tag="cb1")
            nc.sync.dma_start(cb1[:], d_cb1[:])
            cwms = cp.tile([128, 768], bf16, tag="cwms")
            nc.sync.dma_start(cwms[:], d_cwms[:])
            cwgs = cp.tile([128, 768], bf16, tag="cwgs")
            nc.sync.dma_start(cwgs[:], d_cwgs[:])
            cwlin = cp.tile([128, 384], bf16, tag="cwlin")
            nc.sync.dma_start(cwlin[:], d_cwlin[:])
            crid = cp.tile([5, 384], bf16, tag="crid")
            nc.sync.dma_start(crid[:], d_crid[:])

            # ---- initial embed ----
            fea = fp.tile([128, N], f32, tag="fea")
            for c in range(N // 512):
                sl = slice(c * 512, (c + 1) * 512)
                emb_ps = wps.tile([63, 512], f32, tag="w2")
                nc.tensor.matmul(emb_ps[:], (wi[0:128, 0:63]), (ef1[:, sl]),
                                 start=True, stop=False)
                nc.tensor.matmul(emb_ps[:], (wi[0:72, 63:126]), (ef2[:, sl]),
                                 start=False, stop=True)
                nc.scalar.activation(fea[0:63, sl], emb_ps[:], AF.Identity,
                                     bias=binit[:], scale=1.0)
            nc.sync.dma_start(fea[63:64, :], wrow[:])
            nc.sync.dma_start(fea[64:128, :], fea[0:64, :])

            U_OF_T = {0: 0, 1: 1, 4: 2, 5: 3, 2: 4, 3: 5, 6: 6, 7: 7,
                      8: 8, 9: 9, 10: 10, 11: 11}

            def mslice(t, nh, sbe):
                return ("M", U_OF_T[t] * sbe + nh * 512)

            def emit_front(sbi, fea_t, w1t, b1t, state, sbe, kdim):
                """pair build + W1 matmuls + m/m' for superblock sbi."""
                pair = sp.tile([128, sbe], bf16, tag="pair", bufs=3)
                if kdim == 128:
                    colb = sbi * (sbe // K)
                    ng = sbe // 256
                    self_src = (fea_t[0:64, colb:colb + ng * K]
                                .rearrange("p (g i) -> p g i", g=ng)
                                .unsqueeze(3).broadcast_to([64, ng, K, K]))
                    nbr_src = (fea_t[64:128, colb:colb + ng * K]
                               .rearrange("p (g j) -> p g j", g=ng)
                               .unsqueeze(2).broadcast_to([64, ng, K, K]))
                    nc.vector.tensor_copy(
                        pair[0:64, :].rearrange("p (g i j) -> p g i j", g=ng, i=K),
                        self_src)
                    nc.scalar.activation(
                        pair[64:128, :].rearrange("p (g i j) -> p g i j", g=ng, i=K),
                        nbr_src, AF.Identity)
                else:
                    nc.vector.tensor_copy(pair[0:64, :], fea_t[0:64, :])
                mAB = sp.tile([128, NT * sbe], bf16, tag="mAB", bufs=3)
                for t in sorted(range(NT), key=lambda t: U_OF_T[t]):
                    u = U_OF_T[t]
                    pre = pps.tile([128, sbe], f32, tag="pre")
                    for nh in range(sbe // 512):
                        nc.tensor.matmul(pre[:, nh * 512:nh * 512 + 512],
                                         (w1t[0:kdim, t * 128:t * 128 + 128]),
                                         (pair[0:kdim, nh * 512:nh * 512 + 512]),
                                         start=True, stop=True)
                    if t < NDM:
                        nc.vector.tensor_scalar(
                            mAB[:, u * sbe:(u + 1) * sbe], pre[:],
                            b1t[:, t:t + 1], 0.0, op0=ALU.add, op1=ALU.min)
                    else:
                        nc.scalar.activation(
                            mAB[:, u * sbe:(u + 1) * sbe], pre[:],
                            AF.Relu, bias=b1t[:, 12 + (t - NDM):13 + (t - NDM)],
                            scale=-1.0)
                state[sbi] = (pair, mAB, None)

            def emit_vs(sbi, state, sbe):
                """wide-batched v = e^m for superblock sbi."""
                pair, mAB, _ = state[sbi]
                vS = sp.tile([128, NT * sbe], bf16, tag="vS", bufs=2)
                nc.scalar.activation(vS[:, 0:4 * sbe], mAB[:, 0:4 * sbe],
                                     AF.Exp)
                nc.scalar.activation(vS[:, 4 * sbe:8 * sbe],
                                     mAB[:, 4 * sbe:8 * sbe], AF.Exp)
                nc.scalar.activation(vS[:, 8 * sbe:9 * sbe],
                                     mAB[:, 8 * sbe:9 * sbe], AF.Exp)
                nc.scalar.activation(vS[:, 9 * sbe:12 * sbe],
                                     mAB[:, 9 * sbe:12 * sbe], AF.Exp,
                                     scale=-1.0)
                state[sbi] = (pair, mAB, vS)

            def emit_back(sbi, state, rider, wmst, wgst, wlint, ridt,
                          dn01, rn01, dn2, rn2, sbe, kdim, fin=None):
                """W2 matmuls + softmax + segmented reduces for superblock sbi."""
                pair, mAB, vS = state.pop(sbi)
                nhs = sbe // 512

                def rhs(kind, off):
                    src = mAB if kind == "M" else vS
                    return src[:, off:off + 512]

                def stacked(ps, h, mlp, base, tp):
                    """8 accumulating chunk matmuls for head h of mlp into
                    row-block tp of ps (per nh bank)."""
                    rows = slice(64, 128) if tp else slice(0, 64)
                    wt = wgst if mlp == 0 else wmst
                    for nh in range(nhs):
                        for k in range(4):
                            half = k % 2
                            t = h * 4 + mlp * 2 + half
                            if k < 2:
                                kind, off = "V", U_OF_T[t] * sbe + nh * 512
                            else:
                                kind, off = mslice(t, nh, sbe)
                            last = (k == 3)
                            nc.tensor.matmul(
                                ps[rows, nh * 512:nh * 512 + 512],
                                (wt[:, h * 256 + k * 64:h * 256 + k * 64 + 64]),
                                (rhs(kind, off)),
                                start=False, stop=(last and base),
                                tile_position=((0, 64) if tp else None),
                                skip_group_check=True)

                def headpair(rcol, lcol, hs, mlps, esl0):
                    """one W2 PSUM tile: rider + linear + 2 row-blocks."""
                    ps = wps.tile([128, sbe], f32, tag="w2")
                    for nh in range(nhs):
                        esl = slice(esl0 + sbi * sbe + nh * 512,
                                    esl0 + sbi * sbe + nh * 512 + 512)
                        nc.tensor.matmul(ps[:, nh * 512:nh * 512 + 512],
                                         (ridt[0:5, rcol:rcol + 128]),
                                         (rider[:, esl]),
                                         start=True, stop=False,
                                         skip_group_check=True)
                        nc.tensor.matmul(ps[:, nh * 512:nh * 512 + 512],
                                         (wlint[0:kdim, lcol:lcol + 128]),
                                         (pair[0:kdim, nh * 512:nh * 512 + 512]),
                                         start=False, stop=False,
                                         skip_group_check=True)
                    stacked(ps, hs[0], mlps[0], False, False)
                    stacked(ps, hs[1], mlps[1], True, True)
                    return ps

                gate01 = headpair(0, 0, (0, 1), (0, 0), 0)
                msg01 = headpair(128, 128, (0, 1), (1, 1), 0)
                g2m2 = headpair(256, 256, (2, 2), (1, 0), 0)

                nseg = sbe // K
                seg = slice(sbi * nseg, (sbi + 1) * nseg)
                z01 = zp.tile([128, sbe], bf16, tag="z01")
                nc.scalar.activation(z01[:], gate01[:], AF.Exp)
                nc.vector.tensor_reduce(
                    out=dn01[:, seg],
                    in_=z01[:].rearrange("p (s j) -> p s j", j=K),
                    axis=AX.X, op=ALU.add)
                prod01 = zp.tile([128, sbe], bf16, tag="prod01")
                nc.vector.tensor_tensor(out=prod01[:], in0=msg01[:], in1=z01[:],
                                        op=ALU.mult)
                nc.vector.tensor_reduce(
                    out=rn01[:, seg],
                    in_=prod01[:].rearrange("p (s j) -> p s j", j=K),
                    axis=AX.X, op=ALU.add)
                z2 = zp.tile([64, sbe], bf16, tag="z2")
                nc.scalar.activation(z2[:], g2m2[64:128, :], AF.Exp)
                nc.vector.tensor_reduce(
                    out=dn2[:, seg],
                    in_=z2[:].rearrange("p (s j) -> p s j", j=K),
                    axis=AX.X, op=ALU.add)
                prod2 = zp.tile([64, sbe], bf16, tag="prod2")
                nc.vector.tensor_tensor(out=prod2[:], in0=g2m2[0:64, :],
                                        in1=z2[:], op=ALU.mult)
                nc.vector.tensor_reduce(
                    out=rn2[:, seg],
                    in_=prod2[:].rearrange("p (s j) -> p s j", j=K),
                    axis=AX.X, op=ALU.add)
                if fin is not None:
                    fea_src, fea_dst = fin
                    nc.vector.reciprocal(dn01[:, seg], dn01[:, seg])
                    nc.vector.reciprocal(dn2[:, seg], dn2[:, seg])
                    nc.vector.tensor_tensor(out=rn01[:, seg], in0=rn01[:, seg],
                                            in1=dn01[:, seg], op=ALU.mult)
                    nc.vector.tensor_tensor(out=rn2[:, seg], in0=rn2[:, seg],
                                            in1=dn2[:, seg], op=ALU.mult)
                    u1lo = np_.tile([64, nseg], f32, tag="u1lo", bufs=3)
                    nc.sync.dma_start(u1lo[:], rn01[64:128, seg])
                    nc.gpsimd.tensor_tensor(out=rn2[:, seg], in0=rn2[:, seg],
                                            in1=u1lo[:], op=ALU.add)
                    nc.gpsimd.tensor_tensor(out=rn2[:, seg], in0=rn2[:, seg],
                                            in1=rn01[0:64, seg], op=ALU.add)
                    nc.gpsimd.tensor_tensor(out=fea_dst[0:64, seg],
                                            in0=fea_src[0:64, seg],
                                            in1=rn2[:, seg], op=ALU.add)
                    nc.sync.dma_start(fea_dst[64:128, seg], fea_dst[0:64, seg])

            def finish_update(dn01, rn01, dn2, rn2, nseg):
                nc.vector.reciprocal(dn01[:], dn01[:])
                nc.vector.reciprocal(dn2[:], dn2[:])
                nc.vector.tensor_tensor(out=rn01[:], in0=rn01[:], in1=dn01[:],
                                        op=ALU.mult)
                nc.vector.tensor_tensor(out=rn2[:], in0=rn2[:], in1=dn2[:],
                                        op=ALU.mult)
                upd1lo = np_.tile([64, nseg], f32, tag="upd1lo")
                nc.sync.dma_start(upd1lo[:], rn01[64:128, :])
                nc.vector.tensor_tensor(out=rn2[:], in0=rn2[:], in1=upd1lo[:],
                                        op=ALU.add)
                nc.vector.tensor_tensor(out=rn2[:], in0=rn2[:], in1=rn01[0:64, :],
                                        op=ALU.add)
                return rn2

            # ---------------- message passing layers ----------------
            for l in range(L):
                dn01 = np_.tile([128, N], f32, tag="dn01")
                rn01 = np_.tile([128, N], f32, tag="rn01")
                dn2 = np_.tile([64, N], f32, tag="dn2")
                rn2 = np_.tile([64, N], f32, tag="rn2")
                fea2 = fp.tile([128, N], f32, tag="fea")
                state = {}
                emit_front(0, fea, w1s[l], b1s[l], state, SBE, 128)
                emit_vs(0, state, SBE)
                emit_front(1, fea, w1s[l], b1s[l], state, SBE, 128)
                for sbi in range(SB):
                    emit_back(sbi, state, re5, wmss[l], wgss[l],
                              wlins[l], rids[l], dn01, rn01, dn2, rn2,
                              SBE, 128, fin=(fea, fea2))
                    if sbi + 1 < SB:
                        emit_vs(sbi + 1, state, SBE)
                    if sbi + 2 < SB:
                        emit_front(sbi + 2, fea, w1s[l], b1s[l], state,
                                   SBE, 128)
                fea = fea2

            # ---------------- crystal pooling ----------------
            dn01 = np_.tile([128, GPC], f32, tag="dn01")
            rn01 = np_.tile([128, GPC], f32, tag="rn01")
            dn2 = np_.tile([64, GPC], f32, tag="dn2")
            rn2 = np_.tile([64, GPC], f32, tag="rn2")
            state = {}
            emit_front(0, fea, cw1, cb1, state, 512, 64)
            emit_vs(0, state, 512)
            emit_back(0, state, rc5, cwms, cwgs, cwlin, crid,
                      dn01, rn01, dn2, rn2, 512, 64)
            cry = finish_update(dn01, rn01, dn2, rn2, GPC)
            nc.sync.dma_start(d_out[:], cry[:])

    nc.compile()
    return nc


def _prep_core_inputs(core, elem_weights, elem_fea_in, W_init, b_init,
                      mg_W1, mg_b1, mg_W2, mg_b2, mm_W1, mm_b1, mm_W2, mm_b2,
                      m_pow, cg_W1, cg_b1, cg_W2, cg_b2, cm_W1, cm_b1, cm_W2,
                      cm_b2, c_pow):
    import ml_dtypes
    f = np.float32
    bf = ml_dtypes.bfloat16
    n0 = core * N
    w = np.ascontiguousarray(elem_weights[n0:n0 + N]).astype(f)
    ef = np.ascontiguousarray(elem_fea_in[n0:n0 + N]).astype(f)

    ins = {}
    ins["eft"] = np.ascontiguousarray(ef.T).astype(bf)
    wi = np.zeros((128, 126), f)
    wi[0:128, 0:63] = W_init[0:128]
    wi[0:72, 63:126] = W_init[128:200]
    ins["wipack"] = wi.astype(bf)
    ins["binit"] = b_init.reshape(63, 1).astype(f)
    ins["wrow"] = w.reshape(1, N)

    def hilo(x64):
        hi = x64.astype(bf).astype(np.float64)
        lo = (x64 - hi).astype(bf)
        return hi.astype(bf), lo

    # edge rider rows: [hiE, loE, hiE, maskE, onesE]
    j_of_e = np.tile(np.arange(K), GPC * K)
    gi_of_e = np.repeat(np.arange(GPC * K), K)
    g_of_e = gi_of_e // K
    i_of_e = gi_of_e % K
    wn = w[g_of_e * K + j_of_e].astype(np.float64)
    hiE, loE = hilo(np.log(wn))
    maskE = np.where(i_of_e == j_of_e, MASKNEG, 0.0)
    re5 = np.zeros((5, E), bf)
    re5[0] = hiE; re5[1] = loE; re5[2] = hiE
    re5[3] = maskE.astype(bf); re5[4] = 1.0
    ins["riderE5"] = re5
    hiC, loC = hilo(np.log(w.astype(np.float64)))
    rc5 = np.zeros((5, N), bf)
    rc5[0] = hiC; rc5[1] = loC; rc5[2] = hiC
    rc5[3] = 0.0; rc5[4] = 1.0
    ins["riderC5"] = rc5

    def pack_wap(W1g, b1g, W2g, b2g, W1m, b1m, W2m, b2m, pw, indim):
        """W1g/W1m: [H, indim(2F or F), HID]; W2g: [H,HID]; W2m: [H,HID,F];
        b2m: [H,F]; b2g: [H]; pw: [H]. Returns the packed tensors."""
        w1 = np.zeros((128, 1536), f)
        b1 = np.zeros((128, 24), f)
        wms = np.zeros((128, 768), f)
        wgs = np.zeros((128, 768), f)
        wlin = np.zeros((128, 384), f)
        rid = np.zeros((5, 384), np.float64)
        for h in range(H):
            for mlp, (W1x, b1x) in enumerate(((W1g[h], b1g[h]), (W1m[h], b1m[h]))):
                for half in range(2):
                    t = h * 4 + mlp * 2 + half
                    w1[0:indim, t * 128:(t + 1) * 128] = \
                        W1x[:, half * 128:(half + 1) * 128]
                    b1[:, t] = b1x[half * 128:(half + 1) * 128]
                    if t >= NDM:
                        b1[:, 12 + (t - NDM)] = -b1x[half * 128:(half + 1) * 128]
            for k in range(4):
                half = k % 2
                hsl = slice(half * 128, (half + 1) * 128)
                col = slice(h * 256 + k * 64, h * 256 + k * 64 + 64)
                tg = h * 4 + 0 * 2 + half
                tm = h * 4 + 1 * 2 + half
                if k < 2:
                    wms[:, col] = (LAM * ALPHA / H) * W2m[h][hsl]
                    wgs[:, col] = np.repeat(
                        ((LAM * ALPHA) * W2g[h][hsl])[:, None], 64, 1)
                else:
                    sgm = -1.0 if tm < NDM else 1.0
                    sgg = -1.0 if tg < NDM else 1.0
                    wms[:, col] = sgm * (LAM / H) * W2m[h][hsl]
                    wgs[:, col] = np.repeat(
                        (sgg * LAM * W2g[h][hsl])[:, None], 64, 1)
            # linear path + consts
            glin = LAM * (W1g[h] @ W2g[h])                       # [indim]
            mlin = (LAM / H) * (W1m[h] @ W2m[h])                 # [indim, F]
            gconst = b2g[h] - LAM * ALPHA * W2g[h].sum() + LAM * (b1g[h] @ W2g[h])
            mconst = (b2m[h] - LAM * ALPHA * W2m[h].sum(0)
                      + LAM * (b1m[h] @ W2m[h])) / H             # [F]
            pw_hi = np.float64(np.float32(bf(pw[h])))
            pw_lo = np.float64(pw[h]) - pw_hi
            if h < 2:
                cols = slice(h * 64, (h + 1) * 64)
                wlin[0:indim, cols] = np.repeat(glin[:, None], 64, 1)
                wlin[0:indim, 128 + h * 64:128 + (h + 1) * 64] = mlin
                rid[0, cols] = pw_hi; rid[1, cols] = pw_hi
                rid[2, cols] = pw_lo; rid[3, cols] = 1.0
                rid[4, cols] = gconst
                rid[4, 128 + h * 64:128 + (h + 1) * 64] = mconst
            else:
                wlin[0:indim, 256:320] = mlin
                wlin[0:indim, 320:384] = np.repeat(glin[:, None], 64, 1)
                rid[4, 256:320] = mconst
                rid[0, 320:384] = pw_hi; rid[1, 320:384] = pw_hi
                rid[2, 320:384] = pw_lo; rid[3, 320:384] = 1.0
                rid[4, 320:384] = gconst
        return (w1.astype(bf), b1, wms.astype(bf), wgs.astype(bf),
                wlin.astype(bf), rid.astype(f).astype(bf))

    for l in range(L):
        w1, b1, wms, wgs, wlin, rid = pack_wap(
            mg_W1[l], mg_b1[l], mg_W2[l], mg_b2[l],
            mm_W1[l], mm_b1[l], mm_W2[l], mm_b2[l], m_pow[l], 2 * F)
        ins[f"w1pack{l}"] = w1
        ins[f"b1pack{l}"] = b1
        ins[f"wms{l}"] = wms
        ins[f"wgs{l}"] = wgs
        ins[f"wlin{l}"] = wlin
        ins[f"rid{l}"] = rid

    w1, b1, wms, wgs, wlin, rid = pack_wap(
        cg_W1, cg_b1, cg_W2, cg_b2, cm_W1, cm_b1, cm_W2, cm_b2, c_pow, F)
    ins["cw1pack"] = w1
    ins["cb1pack"] = b1
    ins["cwms"] = wms
    ins["cwgs"] = wgs
    ins["cwlin"] = wlin
    ins["crid"] = rid
    return {k: np.ascontiguousarray(v) for k, v in ins.items()}


def _check_structure(batch, self_idx, nbr_idx):
    exp_batch = np.repeat(np.arange(G, dtype=np.int64), K)
    i = np.arange(K)
    src, dst = np.meshgrid(i, i, indexing="ij")
    m = src != dst
    offs = (np.arange(G) * K)[:, None]
    exp_self = (offs + src[m][None, :]).reshape(-1)
    exp_nbr = (offs + dst[m][None, :]).reshape(-1)
    if not (np.array_equal(np.asarray(batch, np.int64), exp_batch)
            and np.array_equal(np.asarray(self_idx, np.int64), exp_self)
            and np.array_equal(np.asarray(nbr_idx, np.int64), exp_nbr)):
        raise NotImplementedError(
            "kernel specialized to the 256x16 fully-connected mesh structure")


def kernel(**inputs):
    from concourse.bass_utils import run_bass_kernel_spmd

    _check_structure(inputs["batch"], inputs["self_idx"], inputs["nbr_idx"])
    args = {k: np.asarray(v) for k, v in inputs.items()
            if k not in ("batch", "self_idx", "nbr_idx")}

    if "nc" not in _PROGRAM_CACHE:
        _PROGRAM_CACHE["nc"] = _build_program()
    nc = _PROGRAM_CACHE["nc"]

    in_maps = [_prep_core_inputs(c, **args) for c in range(NCORES)]
    res = run_bass_kernel_spmd(nc, in_maps, list(range(NCORES)))
    out = np.concatenate([res.results[c]["out"].T for c in range(NCORES)], axis=0)
    return out.astype(np.float32)
